# revision 7
# baseline (speedup 1.0000x reference)
"""Trainium2 Bass kernel for nn_ClassAttentionBlock (8-core SPMD).

Pipeline per core (8 of 64 batches, data-parallel):
  phase 1: grouped 3x3 conv via K=96 (ci,dy)-packed matmuls over 3 dx taps,
           BN partial stats via bn_stats on PSUM
  sync:    AllReduce of per-channel (sum, sumsq) across the 8 cores
  phase 2: fused BN(scale,bias)+ReLU, grouped 1x1 qkv projections
           (q in transposed [s, ch] layout; k restricted to the class band,
           v band in [ch, s] layout via host-scattered weight tensors)
  phase 3: dotT = k_band^T q over spatial, exp blocks, Z via ones-matmul,
           o = E^T v scaled by 1/Z on the output copy

The class mask never materializes: softmax columns outside the class band
are exactly zero, so only the 16-wide band is computed.  y-dependent channel
selection is folded into host-gathered weight tensors so the NEFF is static.
Host side: shard batch, pack weights, assemble attn from the E/Z outputs.
"""

import numpy as np
import ml_dtypes

import concourse.bass as bass
import concourse.bacc as bacc
import concourse.mybir as mybir
import concourse.tile as tile
from concourse.bass_utils import run_bass_kernel_spmd

N_CORES = 8
BL = 8            # batches per core
GROUPS = 10
S = 1024          # 32*32 spatial
NH = 4
D = 160
SCALE = 640.0 ** (-0.5)
BN_EPS = 1e-5
F32 = mybir.dt.float32
BF16 = mybir.dt.bfloat16

# mm: 'f32' | 'f32r' | 'bf16'   (matmul operand dtype for the big matmuls)
# resident: keep conv output in SBUF (bf16) instead of spilling f32 to DRAM
DEFAULT_CFG = ("f32r", True)


# ----------------------------------------------------------------- builder --
def build_nc(mm: str, resident: bool):
    F32R = mybir.dt.float32r
    # storage dtype for every tensor that feeds a matmul: walrus requires
    # fp32r matmul inputs to be rounded fp32r at the producer, so the whole
    # chain carries the dtype (same 4-byte layout as f32 host-side).
    dt_io = {"f32": F32, "f32r": F32R, "bf16": BF16}[mm]
    np_io = ml_dtypes.bfloat16 if mm == "bf16" else np.float32
    dt_res = BF16 if resident else F32          # conv-out storage

    nc = bacc.Bacc("TRN2", target_bir_lowering=False, debug=False,
                   num_devices=N_CORES)

    x_in = nc.dram_tensor("x_l", [BL, 320, S], dt_io, kind="ExternalInput").ap()
    w1r_in = nc.dram_tensor("w1r", [96, 1920], dt_io, kind="ExternalInput").ap()
    wq_in = nc.dram_tensor("wq", [128, 768], dt_io, kind="ExternalInput").ap()
    wk_in = nc.dram_tensor("wk", [BL, 128, 320], dt_io, kind="ExternalInput").ap()
    wv_in = nc.dram_tensor("wv", [BL, 128, 320], dt_io, kind="ExternalInput").ap()
    gma_in = nc.dram_tensor("gma", [128, 5], F32, kind="ExternalInput").ap()
    mask_in = nc.dram_tensor("mask", [64, 640], F32, kind="ExternalInput").ap()
    zpad_in = nc.dram_tensor("zpad", [96, 64], dt_io, kind="ExternalInput").ap()
    bta_in = nc.dram_tensor("bta", [128, 5], F32, kind="ExternalInput").ap()

    o_out = nc.dram_tensor("o_out", [BL, 640, S], F32, kind="ExternalOutput").ap()
    eT_out = nc.dram_tensor("eT_out", [BL, 64, 640], F32, kind="ExternalOutput").ap()
    z_out = nc.dram_tensor("z_out", [BL, 128, 5], F32, kind="ExternalOutput").ap()

    if not resident:
        cv_dram = nc.dram_tensor("cv_spill", [BL, 5, 128, S], F32).ap()

    def mc(ap, n):
        # fp32r runs 1 cyc/row when the moving dim >= 256, else no better
        # than fp32 -- keep plain fp32 numerics for narrow matmuls.
        if mm == "f32r" and n >= 256:
            return ap.bitcast(mybir.dt.float32r)
        return ap

    with tile.TileContext(nc) as tc:
        with (
            tc.tile_pool(name="persist", bufs=1) as pp,
            tc.tile_pool(name="work", bufs=2) as wp,
            tc.tile_pool(name="psA", bufs=2, space="PSUM") as psA,
            tc.tile_pool(name="psB", bufs=4, space="PSUM") as psB,
            tc.tile_pool(name="dram", bufs=1, space="DRAM") as dp,
        ):
            # ---- persistent weights / constants
            w1r_sb = pp.tile([96, 1920], dt_io, tag="w1r")
            nc.sync.dma_start(w1r_sb, w1r_in)
            wq_sb = pp.tile([128, 768], dt_io, tag="wq")
            nc.sync.dma_start(wq_sb, wq_in)
            gma_sb = pp.tile([128, 5], F32, tag="gma")
            nc.sync.dma_start(gma_sb, gma_in)
            bta_sb = pp.tile([128, 5], F32, tag="bta")
            nc.sync.dma_start(bta_sb, bta_in)
            dt_ones = BF16 if mm == "bf16" else F32
            ones_sb = pp.tile([64, 1], dt_ones, tag="ones")
            nc.gpsimd.memset(ones_sb, 1.0)

            # xrep3 double buffer: [96, 34*32] padded rows (col 0 and 33 are
            # the zero padding), partition p = 32*dy + ci holds row-shifted x.
            # Borders zero-filled once via DMA (f32r has no memset).
            xreps = []
            for i in range(2):
                xr = pp.tile([96, 34 * 32], dt_io, tag=f"xrep{i}",
                             name=f"xrep{i}")
                xr3 = xr.rearrange("p (h c) -> p h c", c=34)
                nc.sync.dma_start(xr3[0:32, 0, :], zpad_in[0:32, 0:34])
                nc.sync.dma_start(xr3[64:96, 31, :], zpad_in[64:96, 0:34])
                nc.sync.dma_start(xr3[:, :, 0:1], zpad_in[:, 0:32].rearrange(
                    "p (a b) -> p a b", b=1))
                nc.sync.dma_start(xr3[:, :, 33:34], zpad_in[:, 32:64].rearrange(
                    "p (a b) -> p a b", b=1))
                xreps.append(xr)

            # E_T double buffer (fully overwritten per batch) + band mask
            ets = [pp.tile([64, 640], F32, tag=f"et{i}", name=f"et{i}")
                   for i in range(2)]
            if mm != "f32":
                etbs = [pp.tile([64, 640], dt_io, tag=f"etb{i}",
                                name=f"etb{i}") for i in range(2)]
            mask_sb = pp.tile([64, 640], F32, tag="mask")
            nc.sync.dma_start(mask_sb, mask_in)

            # conv-out residency
            if resident:
                convout = [[pp.tile([128, S], dt_res, tag=f"cv{b}_{t}",
                                    name=f"cv{b}_{t}")
                            for t in range(5)] for b in range(BL)]

            stats_sb = pp.tile([128, 480], F32, tag="stats")
            mv_sb = pp.tile([128, 10], F32, tag="mv")
            scale_sb = pp.tile([128, 5], F32, tag="scale")
            bias_sb = pp.tile([128, 5], F32, tag="bias")
            rz_sb = pp.tile([128, 5], F32, tag="rz")

            # per-parity persistent qkv products
            qts = [[pp.tile([128, 640], dt_io, tag=f"qt{c}_{i}",
                            name=f"qt{c}_{i}") for c in range(8)]
                   for i in range(2)]
            kts = [pp.tile([128, 512], dt_io, tag=f"kt{i}", name=f"kt{i}")
                   for i in range(2)]
            vs = [pp.tile([64, S], dt_io, tag=f"v{i}", name=f"v{i}")
                  for i in range(2)]

            cc_in = dp.tile([128, 10], F32, tag="cc_in")
            cc_out = dp.tile([128, 10], F32, tag="cc_out", addr_space="Shared")

            # ================= phase 1: conv + stats =================
            for b in range(BL):
                for g in range(GROUPS):
                    xr = xreps[(b * GROUPS + g) % 2]
                    xs = x_in[b, 32 * g:32 * g + 32, :].rearrange(
                        "p (h w) -> p h w", w=32)
                    xr3 = xr.rearrange("p (h c) -> p h c", c=34)
                    # dy-shifted copies: xr3[32*dy+ci, r, 1+w] = x[ci, r+dy-1, w]
                    nc.sync.dma_start(xr3[0:32, 1:32, 1:33], xs[:, 0:31, :])
                    nc.sync.dma_start(xr3[32:64, :, 1:33], xs)
                    nc.sync.dma_start(xr3[64:96, 0:31, 1:33], xs[:, 1:32, :])

                    ps = psA.tile([64, S], F32, tag="big", name="cps")
                    xv = xr.rearrange("p (h c) -> p h c", c=34)
                    for half in range(2):
                        h0 = 16 * half
                        for dx in (0, 1, 2):
                            lhsT = w1r_sb[:, (3 * g + dx) * 64:(3 * g + dx) * 64 + 64]
                            rhs = xv[:, h0:h0 + 16, dx:dx + 32]
                            nc.tensor.matmul(
                                ps[:, 512 * half:512 * half + 512], lhsT, rhs,
                                start=(dx == 0), stop=(dx == 2))
                    gg, t = g % 2, g // 2
                    for half in range(2):
                        nc.vector.bn_stats(
                            stats_sb[64 * gg:64 * gg + 64,
                                     96 * t + 6 * (2 * b + half):
                                     96 * t + 6 * (2 * b + half) + 6],
                            ps[:, 512 * half:512 * half + 512])
                    if resident:
                        nc.scalar.copy(convout[b][t][64 * gg:64 * gg + 64, :], ps)
                    else:
                        cvt = wp.tile([64, S], F32, tag="cvstage", bufs=3,
                                      name="cvstage")
                        nc.scalar.copy(cvt, ps)
                        nc.sync.dma_start(
                            cv_dram[b, t, 64 * gg:64 * gg + 64, :], cvt)

            # ================= stats reduce + BN coefficients ========
            for t in range(5):
                triples = stats_sb[:, 96 * t:96 * t + 96].rearrange(
                    "p (pair eo three) -> p pair eo three", eo=2, three=3)
                nc.vector.bn_aggr(mv_sb[:, 2 * t:2 * t + 2], triples)
            mvv = mv_sb.rearrange("p (t two) -> p two t", two=2)
            means, vars_ = mvv[:, 0, :], mvv[:, 1, :]
            cc_sb = pp.tile([128, 10], F32, tag="cc_sb")
            sq_sb = pp.tile([128, 5], F32, tag="sq")
            nc.vector.tensor_mul(sq_sb, means, means)
            nc.vector.tensor_add(sq_sb, sq_sb, vars_)
            nc.scalar.mul(cc_sb[:, 0:5], means, 8192.0)
            nc.scalar.mul(cc_sb[:, 5:10], sq_sb, 8192.0)
            nc.sync.dma_start(cc_in, cc_sb)
            nc.gpsimd.collective_compute(
                "AllReduce", mybir.AluOpType.add,
                replica_groups=[list(range(N_CORES))],
                ins=[cc_in.opt()], outs=[cc_out.opt()])
            tot_sb = pp.tile([128, 10], F32, tag="tot")
            nc.sync.dma_start(tot_sb, cc_out)
            meanT = pp.tile([128, 5], F32, tag="meanT")
            varT = pp.tile([128, 5], F32, tag="varT")
            nc.scalar.mul(meanT, tot_sb[:, 0:5], 1.0 / 65536.0)
            nc.scalar.mul(varT, tot_sb[:, 5:10], 1.0 / 65536.0)
            m2 = pp.tile([128, 5], F32, tag="m2")
            nc.vector.tensor_mul(m2, meanT, meanT)
            nc.vector.tensor_sub(varT, varT, m2)
            nc.vector.tensor_scalar_add(varT, varT, BN_EPS)
            stdT = pp.tile([128, 5], F32, tag="stdT")
            nc.scalar.sqrt(stdT, varT)
            rstdT = pp.tile([128, 5], F32, tag="rstdT")
            nc.vector.reciprocal(rstdT, stdT)
            nc.vector.tensor_mul(scale_sb, rstdT, gma_sb)
            nc.vector.tensor_mul(m2, meanT, scale_sb)
            nc.vector.tensor_sub(bias_sb, bta_sb, m2)

            # ================= phase 2+3 per batch ===================
            for b in range(BL):
                pb = b % 2
                wk_sb = wp.tile([128, 320], dt_io, tag="wk", name="wk_sb")
                nc.sync.dma_start(wk_sb, wk_in[b])
                wv_sb = wp.tile([128, 320], dt_io, tag="wv", name="wv_sb")
                nc.sync.dma_start(wv_sb, wv_in[b])

                bnt = []
                for t in range(5):
                    bn_ = wp.tile([128, S], dt_io, tag=f"bn{t}", bufs=1,
                                  name=f"bn_{t}")
                    if resident:
                        src = convout[b][t]
                    else:
                        src = wp.tile([128, S], F32, tag="rl", bufs=3, name="rl")
                        nc.sync.dma_start(src, cv_dram[b, t])
                    nc.scalar.activation(
                        bn_, src, mybir.ActivationFunctionType.Relu,
                        bias=bias_sb[:, t:t + 1], scale=scale_sb[:, t:t + 1])
                    bnt.append(bn_)

                # q (transposed layout)  psum [s_chunk=128, 640]
                for sc in range(8):
                    qp = psA.tile([128, 640], F32, tag="big", name="qps")
                    sl = slice(128 * sc, 128 * sc + 128)
                    nc.tensor.matmul(qp[:, 0:384], mc(bnt[0][:, sl], 384),
                                     mc(wq_sb[:, 0:384], 384),
                                     start=True, stop=True)
                    nc.tensor.matmul(qp[:, 384:512], bnt[1][:, sl],
                                     wq_sb[:, 384:512], start=True, stop=True)
                    nc.tensor.matmul(qp[:, 512:640], bnt[1][:, sl],
                                     wq_sb[:, 512:640], start=True, stop=True)
                    nc.any.tensor_copy(qts[pb][sc], qp)
                # k band (transposed layout)
                for sc in range(8):
                    kp = psB.tile([128, 64], F32, tag="small", name="kps")
                    sl = slice(128 * sc, 128 * sc + 128)
                    for t in (1, 2, 3):
                        nc.tensor.matmul(kp, bnt[t][:, sl],
                                         wk_sb[:, 64 * t:64 * t + 64],
                                         start=(t == 1), stop=(t == 3))
                    nc.any.tensor_copy(kts[pb][:, 64 * sc:64 * sc + 64], kp)
                # v band (channel layout)
                vp = psA.tile([64, S], F32, tag="big", name="vps")
                for t in range(5):
                    for n_ in range(2):
                        nc.tensor.matmul(
                            vp[:, 512 * n_:512 * n_ + 512],
                            mc(wv_sb[:, 64 * t:64 * t + 64], 512),
                            mc(bnt[t][:, 512 * n_:512 * n_ + 512], 512),
                            start=(t == 0), stop=(t == 4))
                nc.any.tensor_copy(vs[pb], vp)

                # dotT [64 j, 640 i]
                dpm = psA.tile([64, 640], F32, tag="big", name="dps")
                for sc in range(8):
                    ksl = kts[pb][:, 64 * sc:64 * sc + 64]
                    nc.tensor.matmul(dpm[:, 0:512], ksl,
                                     mc(qts[pb][sc][:, 0:512], 512),
                                     start=(sc == 0), stop=(sc == 7))
                    nc.tensor.matmul(dpm[:, 512:640], ksl,
                                     qts[pb][sc][:, 512:640],
                                     start=(sc == 0), stop=(sc == 7))
                # exp on the full tile (engines need 32-aligned partition
                # bases, so per-band slices at partition 16h are illegal);
                # off-band junk is zeroed by the precomputed band mask.
                efull = wp.tile([64, 640], F32, tag="efull", name="efull")
                nc.scalar.activation(efull, dpm,
                                     mybir.ActivationFunctionType.Exp,
                                     scale=SCALE)
                et = ets[pb]
                nc.vector.tensor_mul(et, efull, mask_sb)
                nc.sync.dma_start(eT_out[b], et)
                if mm != "f32":
                    emm = etbs[pb]
                    nc.vector.tensor_copy(emm, et)
                else:
                    emm = et
                # Z via ones-matmul -> [128, 5] per chunk
                zp = psB.tile([128, 8], F32, tag="small", name="zps")
                ez = (et if mm == "f32r" else emm)
                for c in range(5):
                    nc.tensor.matmul(zp[:, c:c + 1],
                                     ez[:, 128 * c:128 * c + 128], ones_sb,
                                     start=True, stop=True)
                nc.vector.reciprocal(rz_sb, zp[:, 0:5])
                zst = wp.tile([128, 5], F32, tag="zst", name="zst")
                nc.vector.tensor_copy(zst, zp[:, 0:5])
                nc.sync.dma_start(z_out[b], zst)
                # o = E^T v * (1/Z)
                for c in range(5):
                    for n_ in range(2):
                        op_ = psB.tile([128, 512], F32, tag="small", name="ops")
                        nc.tensor.matmul(op_,
                                         mc(emm[:, 128 * c:128 * c + 128], 512),
                                         mc(vs[pb][:, 512 * n_:512 * n_ + 512], 512),
                                         start=True, stop=True)
                        ost = wp.tile([128, 512], F32, tag="ost", bufs=3,
                                      name="ost")
                        nc.scalar.mul(ost, op_, rz_sb[:, c:c + 1])
                        nc.sync.dma_start(
                            o_out[b, 128 * c:128 * c + 128,
                                  512 * n_:512 * n_ + 512], ost)

    nc.compile()
    return nc, np_io


# ------------------------------------------------------------- host packing --
def _host_pack(x, y, w1, gamma, beta, w_qkv, np_io):
    B = x.shape[0]
    xf = np.ascontiguousarray(x.reshape(B, 320, S))
    wqkv2 = w_qkv[:, :, 0, 0].astype(np.float32)

    w1r = np.zeros((96, 1920), np.float32)
    for g in range(GROUPS):
        for dx in range(3):
            for dy in range(3):
                w1r[32 * dy:32 * dy + 32,
                    (3 * g + dx) * 64:(3 * g + dx) * 64 + 64] = \
                    w1[64 * g:64 * g + 64, :, dy, dx].T

    wq_bd = np.zeros((128, 768), np.float32)
    for t in range(2):
        for gg in range(2):
            g = 2 * t + gg
            qlo, qhi = 192 * g, min(640, 192 * g + 192)
            if qlo >= qhi:
                continue
            wq_bd[64 * gg:64 * gg + 64,
                  t * 384 + (qlo - 384 * t):t * 384 + (qhi - 384 * t)] = \
                wqkv2[qlo:qhi, :].T

    def band_weights(base):
        wp_ = np.zeros((B, 128, 320), np.float32)
        for b in range(B):
            yb = int(y[b])
            for h in range(NH):
                c0 = base + 160 * h + 16 * yb
                g = c0 // 192
                t, p0 = g // 2, 64 * (g % 2)
                wp_[b, p0:p0 + 64, 64 * t + 16 * h:64 * t + 16 * h + 16] = \
                    wqkv2[c0:c0 + 16, :].T
        return wp_

    wk = band_weights(640)
    wv = band_weights(1280)
    mask_np = np.zeros((64, 640), np.float32)
    for h in range(NH):
        mask_np[16 * h:16 * h + 16, 160 * h:160 * h + 160] = 1.0
    gma = np.ascontiguousarray(gamma.astype(np.float32).reshape(5, 128).T)
    bta = np.ascontiguousarray(beta.astype(np.float32).reshape(5, 128).T)

    in_maps = []
    for k in range(N_CORES):
        sl = slice(BL * k, BL * k + BL)
        in_maps.append({
            "x_l": np.ascontiguousarray(xf[sl]).astype(np_io),
            "w1r": w1r.astype(np_io),
            "wq": wq_bd.astype(np_io),
            "wk": np.ascontiguousarray(wk[sl]).astype(np_io),
            "wv": np.ascontiguousarray(wv[sl]).astype(np_io),
            "gma": gma,
            "mask": mask_np,
            "zpad": np.zeros((96, 64), np_io),
            "bta": bta,
        })
    return in_maps


def _host_assemble(results, y, B):
    o = np.zeros((B, 640, 32, 32), np.float32)
    attn = np.zeros((B, NH, 160, 160), np.float32)
    for k in range(N_CORES):
        r = results[k]
        o[BL * k:BL * k + BL] = r["o_out"].reshape(BL, 640, 32, 32)
        eTf = r["eT_out"]                      # [BL, 64, 640]
        zf = np.transpose(r["z_out"], (0, 2, 1)).reshape(BL, 640)  # [b, ch]
        for lb in range(BL):
            b = BL * k + lb
            yb = int(y[b])
            for h in range(NH):
                zi = zf[lb, 160 * h:160 * h + 160]
                eh = eTf[lb, 16 * h:16 * h + 16, 160 * h:160 * h + 160]
                attn[b, h, :, 16 * yb:16 * yb + 16] = (eh / zi[None, :]).T
    return o, attn


_NC_CACHE = {}


def _get_nc(cfg=DEFAULT_CFG):
    if cfg not in _NC_CACHE:
        _NC_CACHE[cfg] = build_nc(*cfg)
    return _NC_CACHE[cfg]


def run(x, y, w1, b1, gamma, beta, w_qkv, cfg=DEFAULT_CFG, trace=False):
    """Full pipeline; returns ((o, attn), BassKernelResults)."""
    nc, np_io = _get_nc(cfg)
    in_maps = _host_pack(np.asarray(x, np.float32), np.asarray(y),
                         np.asarray(w1, np.float32),
                         np.asarray(gamma, np.float32),
                         np.asarray(beta, np.float32),
                         np.asarray(w_qkv, np.float32), np_io)
    res = run_bass_kernel_spmd(nc, in_maps, list(range(N_CORES)), trace=trace)
    out = _host_assemble(res.results, np.asarray(y), np.asarray(x).shape[0])
    return out, res


def kernel(x, y, w1, b1, gamma, beta, w_qkv):
    out, _ = run(x, y, w1, b1, gamma, beta, w_qkv)
    return out


# revision 9
# speedup vs baseline: 1.3869x; 1.3869x over previous
"""Trainium2 Bass kernel for nn_ClassAttentionBlock (8-core SPMD).

Pipeline per core (8 of 64 batches, data-parallel):
  phase 1: grouped 3x3 conv via K=96 (ci,dy)-packed matmuls over 3 dx taps,
           BN partial stats via bn_stats on PSUM
  sync:    AllReduce of per-channel (sum, sumsq) across the 8 cores
  phase 2: fused BN(scale,bias)+ReLU, grouped 1x1 qkv projections
           (q in transposed [s, ch] layout; k restricted to the class band,
           v band in [ch, s] layout via host-scattered weight tensors)
  phase 3: dotT = k_band^T q over spatial, exp blocks, Z via ones-matmul,
           o = E^T v scaled by 1/Z on the output copy

The class mask never materializes: softmax columns outside the class band
are exactly zero, so only the 16-wide band is computed.  y-dependent channel
selection is folded into host-gathered weight tensors so the NEFF is static.
Host side: shard batch, pack weights, assemble attn from the E/Z outputs.
"""

import numpy as np
import ml_dtypes

import concourse.bass as bass
import concourse.bacc as bacc
import concourse.mybir as mybir
import concourse.tile as tile
from concourse.bass_utils import run_bass_kernel_spmd

N_CORES = 8
BL = 8            # batches per core
GROUPS = 10
S = 1024          # 32*32 spatial
NH = 4
D = 160
SCALE = 640.0 ** (-0.5)
BN_EPS = 1e-5
F32 = mybir.dt.float32
BF16 = mybir.dt.bfloat16

# mm: 'f32' | 'f32r' | 'bf16'   (matmul operand dtype for the big matmuls)
# resident: keep conv output in SBUF (bf16) instead of spilling f32 to DRAM
DEFAULT_CFG = ("f32r", True)


# ----------------------------------------------------------------- builder --
def build_nc(mm: str, resident: bool):
    F32R = mybir.dt.float32r
    # storage dtype for every tensor that feeds a matmul: walrus requires
    # fp32r matmul inputs to be rounded fp32r at the producer, so the whole
    # chain carries the dtype (same 4-byte layout as f32 host-side).
    dt_io = {"f32": F32, "f32r": F32R, "bf16": BF16}[mm]
    np_io = ml_dtypes.bfloat16 if mm == "bf16" else np.float32
    dt_res = BF16 if resident else F32          # conv-out storage

    nc = bacc.Bacc("TRN2", target_bir_lowering=False, debug=False,
                   num_devices=N_CORES)

    xr_in = nc.dram_tensor("x_r", [BL, GROUPS, 96, 1088], dt_io,
                           kind="ExternalInput").ap()
    w1r_in = nc.dram_tensor("w1r", [96, 1920], dt_io, kind="ExternalInput").ap()
    wq_in = nc.dram_tensor("wq", [128, 768], dt_io, kind="ExternalInput").ap()
    wk_in = nc.dram_tensor("wk", [BL, 128, 320], dt_io, kind="ExternalInput").ap()
    wv_in = nc.dram_tensor("wv", [BL, 128, 320], dt_io, kind="ExternalInput").ap()
    gma_in = nc.dram_tensor("gma", [128, 5], F32, kind="ExternalInput").ap()
    mask_in = nc.dram_tensor("mask", [64, 640], F32, kind="ExternalInput").ap()
    bta_in = nc.dram_tensor("bta", [128, 5], F32, kind="ExternalInput").ap()

    o_out = nc.dram_tensor("o_out", [BL, 640, S], F32, kind="ExternalOutput").ap()
    eT_out = nc.dram_tensor("eT_out", [BL, 64, 640], F32, kind="ExternalOutput").ap()
    z_out = nc.dram_tensor("z_out", [BL, 128, 5], F32, kind="ExternalOutput").ap()

    if not resident:
        cv_dram = nc.dram_tensor("cv_spill", [BL, 5, 128, S], F32).ap()

    def mc(ap, n):
        # fp32r runs 1 cyc/row when the moving dim >= 256, else no better
        # than fp32 -- keep plain fp32 numerics for narrow matmuls.
        if mm == "f32r" and n >= 256:
            return ap.bitcast(mybir.dt.float32r)
        return ap

    with tile.TileContext(nc) as tc:
        with (
            tc.tile_pool(name="persist", bufs=1) as pp,
            tc.tile_pool(name="work", bufs=2) as wp,
            tc.tile_pool(name="psA", bufs=2, space="PSUM") as psA,
            tc.tile_pool(name="psB", bufs=4, space="PSUM") as psB,
            tc.tile_pool(name="dram", bufs=1, space="DRAM") as dp,
        ):
            # ---- persistent weights / constants
            w1r_sb = pp.tile([96, 1920], dt_io, tag="w1r")
            nc.sync.dma_start(w1r_sb, w1r_in)
            wq_sb = pp.tile([128, 768], dt_io, tag="wq")
            nc.sync.dma_start(wq_sb, wq_in)
            gma_sb = pp.tile([128, 5], F32, tag="gma")
            nc.sync.dma_start(gma_sb, gma_in)
            bta_sb = pp.tile([128, 5], F32, tag="bta")
            nc.sync.dma_start(bta_sb, bta_in)
            dt_ones = BF16 if mm == "bf16" else F32
            ones_sb = pp.tile([64, 1], dt_ones, tag="ones")
            nc.gpsimd.memset(ones_sb, 1.0)


            # E_T double buffer (fully overwritten per batch) + band mask
            ets = [pp.tile([64, 640], F32, tag=f"et{i}", name=f"et{i}")
                   for i in range(2)]
            if mm != "f32":
                etbs = [pp.tile([64, 640], dt_io, tag=f"etb{i}",
                                name=f"etb{i}") for i in range(2)]
            mask_sb = pp.tile([64, 640], F32, tag="mask")
            nc.sync.dma_start(mask_sb, mask_in)

            # conv-out residency
            if resident:
                convout = [[pp.tile([128, S], dt_res, tag=f"cv{b}_{t}",
                                    name=f"cv{b}_{t}")
                            for t in range(5)] for b in range(BL)]

            stats_sb = pp.tile([128, 480], F32, tag="stats")
            mv_sb = pp.tile([128, 10], F32, tag="mv")
            scale_sb = pp.tile([128, 5], F32, tag="scale")
            bias_sb = pp.tile([128, 5], F32, tag="bias")
            rz_sb = pp.tile([128, 5], F32, tag="rz")

            # per-parity persistent qkv products
            qts = [pp.tile([128, 640], dt_io, tag=f"qt{c}",
                           name=f"qt{c}") for c in range(8)]
            kts = [pp.tile([128, 512], dt_io, tag=f"kt{i}", name=f"kt{i}")
                   for i in range(2)]
            vs = [pp.tile([64, S], dt_io, tag=f"v{i}", name=f"v{i}")
                  for i in range(2)]

            cc_in = dp.tile([128, 10], F32, tag="cc_in")
            cc_out = dp.tile([128, 10], F32, tag="cc_out", addr_space="Shared")

            # ================= phase 1: conv + stats =================
            for b in range(BL):
                for g in range(GROUPS):
                    xr = wp.tile([96, 34 * 32], dt_io, tag="xrep", bufs=3,
                                 name="xr")
                    nc.sync.dma_start(xr, xr_in[b, g])

                    ps = psA.tile([64, S], F32, tag="big", name="cps")
                    xv = xr.rearrange("p (h c) -> p h c", c=34)
                    for half in range(2):
                        h0 = 16 * half
                        for dx in (0, 1, 2):
                            lhsT = w1r_sb[:, (3 * g + dx) * 64:(3 * g + dx) * 64 + 64]
                            rhs = xv[:, h0:h0 + 16, dx:dx + 32]
                            nc.tensor.matmul(
                                ps[:, 512 * half:512 * half + 512], lhsT, rhs,
                                start=(dx == 0), stop=(dx == 2))
                    gg, t = g % 2, g // 2
                    for half in range(2):
                        nc.vector.bn_stats(
                            stats_sb[64 * gg:64 * gg + 64,
                                     96 * t + 6 * (2 * b + half):
                                     96 * t + 6 * (2 * b + half) + 6],
                            ps[:, 512 * half:512 * half + 512])
                    if resident:
                        nc.scalar.copy(convout[b][t][64 * gg:64 * gg + 64, :], ps)
                    else:
                        cvt = wp.tile([64, S], F32, tag="cvstage", bufs=3,
                                      name="cvstage")
                        nc.scalar.copy(cvt, ps)
                        nc.sync.dma_start(
                            cv_dram[b, t, 64 * gg:64 * gg + 64, :], cvt)

            # ================= stats reduce + BN coefficients ========
            for t in range(5):
                triples = stats_sb[:, 96 * t:96 * t + 96].rearrange(
                    "p (pair eo three) -> p pair eo three", eo=2, three=3)
                nc.vector.bn_aggr(mv_sb[:, 2 * t:2 * t + 2], triples)
            mvv = mv_sb.rearrange("p (t two) -> p two t", two=2)
            means, vars_ = mvv[:, 0, :], mvv[:, 1, :]
            cc_sb = pp.tile([128, 10], F32, tag="cc_sb")
            sq_sb = pp.tile([128, 5], F32, tag="sq")
            nc.vector.tensor_mul(sq_sb, means, means)
            nc.vector.tensor_add(sq_sb, sq_sb, vars_)
            nc.scalar.mul(cc_sb[:, 0:5], means, 8192.0)
            nc.scalar.mul(cc_sb[:, 5:10], sq_sb, 8192.0)
            nc.sync.dma_start(cc_in, cc_sb)
            nc.gpsimd.collective_compute(
                "AllReduce", mybir.AluOpType.add,
                replica_groups=[list(range(N_CORES))],
                ins=[cc_in.opt()], outs=[cc_out.opt()])
            tot_sb = pp.tile([128, 10], F32, tag="tot")
            nc.sync.dma_start(tot_sb, cc_out)
            meanT = pp.tile([128, 5], F32, tag="meanT")
            varT = pp.tile([128, 5], F32, tag="varT")
            nc.scalar.mul(meanT, tot_sb[:, 0:5], 1.0 / 65536.0)
            nc.scalar.mul(varT, tot_sb[:, 5:10], 1.0 / 65536.0)
            m2 = pp.tile([128, 5], F32, tag="m2")
            nc.vector.tensor_mul(m2, meanT, meanT)
            nc.vector.tensor_sub(varT, varT, m2)
            nc.vector.tensor_scalar_add(varT, varT, BN_EPS)
            stdT = pp.tile([128, 5], F32, tag="stdT")
            nc.scalar.sqrt(stdT, varT)
            rstdT = pp.tile([128, 5], F32, tag="rstdT")
            nc.vector.reciprocal(rstdT, stdT)
            nc.vector.tensor_mul(scale_sb, rstdT, gma_sb)
            nc.vector.tensor_mul(m2, meanT, scale_sb)
            nc.vector.tensor_sub(bias_sb, bta_sb, m2)

            # ================= phase 2+3 per batch ===================
            for b in range(BL):
                pb = b % 2
                wk_sb = wp.tile([128, 320], dt_io, tag="wk", name="wk_sb")
                nc.sync.dma_start(wk_sb, wk_in[b])
                wv_sb = wp.tile([128, 320], dt_io, tag="wv", name="wv_sb")
                nc.sync.dma_start(wv_sb, wv_in[b])

                bnt = []
                for t in range(5):
                    bn_ = wp.tile([128, S], dt_io, tag=f"bn{t}", bufs=2,
                                  name=f"bn_{t}")
                    if resident:
                        src = convout[b][t]
                    else:
                        src = wp.tile([128, S], F32, tag="rl", bufs=3, name="rl")
                        nc.sync.dma_start(src, cv_dram[b, t])
                    nc.scalar.activation(
                        bn_, src, mybir.ActivationFunctionType.Relu,
                        bias=bias_sb[:, t:t + 1], scale=scale_sb[:, t:t + 1])
                    bnt.append(bn_)

                # q (transposed layout)  psum [s_chunk=128, 640]
                for sc in range(8):
                    qp = psA.tile([128, 640], F32, tag="big", name="qps")
                    sl = slice(128 * sc, 128 * sc + 128)
                    nc.tensor.matmul(qp[:, 0:384], mc(bnt[0][:, sl], 384),
                                     mc(wq_sb[:, 0:384], 384),
                                     start=True, stop=True)
                    nc.tensor.matmul(qp[:, 384:512], bnt[1][:, sl],
                                     wq_sb[:, 384:512], start=True, stop=True)
                    nc.tensor.matmul(qp[:, 512:640], bnt[1][:, sl],
                                     wq_sb[:, 512:640], start=True, stop=True)
                    nc.any.tensor_copy(qts[sc], qp)
                # k band (transposed layout)
                for sc in range(8):
                    kp = psB.tile([128, 64], F32, tag="small", name="kps")
                    sl = slice(128 * sc, 128 * sc + 128)
                    for t in (1, 2, 3):
                        nc.tensor.matmul(kp, bnt[t][:, sl],
                                         wk_sb[:, 64 * t:64 * t + 64],
                                         start=(t == 1), stop=(t == 3))
                    nc.any.tensor_copy(kts[pb][:, 64 * sc:64 * sc + 64], kp)
                # v band (channel layout)
                vp = psA.tile([64, S], F32, tag="big", name="vps")
                for t in range(5):
                    for n_ in range(2):
                        nc.tensor.matmul(
                            vp[:, 512 * n_:512 * n_ + 512],
                            mc(wv_sb[:, 64 * t:64 * t + 64], 512),
                            mc(bnt[t][:, 512 * n_:512 * n_ + 512], 512),
                            start=(t == 0), stop=(t == 4))
                nc.any.tensor_copy(vs[pb], vp)

                # dotT [64 j, 640 i]
                dpm = psA.tile([64, 640], F32, tag="big", name="dps")
                for sc in range(8):
                    ksl = kts[pb][:, 64 * sc:64 * sc + 64]
                    nc.tensor.matmul(dpm[:, 0:512], ksl,
                                     qts[sc][:, 0:512],
                                     start=(sc == 0), stop=(sc == 7))
                    nc.tensor.matmul(dpm[:, 512:640], ksl,
                                     qts[sc][:, 512:640],
                                     start=(sc == 0), stop=(sc == 7))
                # exp on the full tile (engines need 32-aligned partition
                # bases, so per-band slices at partition 16h are illegal);
                # off-band junk is zeroed by the precomputed band mask.
                efull = wp.tile([64, 640], F32, tag="efull", name="efull")
                nc.scalar.activation(efull, dpm,
                                     mybir.ActivationFunctionType.Exp,
                                     scale=SCALE)
                et = ets[pb]
                nc.vector.tensor_mul(et, efull, mask_sb)
                nc.sync.dma_start(eT_out[b], et)
                if mm != "f32":
                    emm = etbs[pb]
                    nc.vector.tensor_copy(emm, et)
                else:
                    emm = et
                # Z via ones-matmul -> [128, 5] per chunk
                zp = psB.tile([128, 8], F32, tag="small", name="zps")
                ez = (et if mm == "f32r" else emm)
                for c in range(5):
                    nc.tensor.matmul(zp[:, c:c + 1],
                                     ez[:, 128 * c:128 * c + 128], ones_sb,
                                     start=True, stop=True)
                nc.vector.reciprocal(rz_sb, zp[:, 0:5])
                zst = wp.tile([128, 5], F32, tag="zst", name="zst")
                nc.vector.tensor_copy(zst, zp[:, 0:5])
                nc.sync.dma_start(z_out[b], zst)
                # o = E^T v * (1/Z)
                for c in range(5):
                    for n_ in range(2):
                        op_ = psB.tile([128, 512], F32, tag="small", name="ops")
                        nc.tensor.matmul(op_,
                                         mc(emm[:, 128 * c:128 * c + 128], 512),
                                         mc(vs[pb][:, 512 * n_:512 * n_ + 512], 512),
                                         start=True, stop=True)
                        ost = wp.tile([128, 512], F32, tag="ost", bufs=3,
                                      name="ost")
                        nc.scalar.mul(ost, op_, rz_sb[:, c:c + 1])
                        nc.sync.dma_start(
                            o_out[b, 128 * c:128 * c + 128,
                                  512 * n_:512 * n_ + 512], ost)

    nc.compile()
    return nc, np_io


# ------------------------------------------------------------- host packing --
def _host_pack(x, y, w1, gamma, beta, w_qkv, np_io):
    B = x.shape[0]
    # padded dy-replicated conv input: rep[b, g, 32*dy+ci, r, 1+w]
    #   = x[b, 32g+ci, r+dy-1, w], zero padding elsewhere
    x4 = x.reshape(B, GROUPS, 32, 32, 32)
    rep = np.zeros((B, GROUPS, 3, 32, 32, 34), np.float32)
    rep[:, :, 0, :, 1:32, 1:33] = x4[:, :, :, 0:31, :]
    rep[:, :, 1, :, :, 1:33] = x4
    rep[:, :, 2, :, 0:31, 1:33] = x4[:, :, :, 1:32, :]
    rep = rep.reshape(B, GROUPS, 96, 1088)
    wqkv2 = w_qkv[:, :, 0, 0].astype(np.float32)

    w1r = np.zeros((96, 1920), np.float32)
    for g in range(GROUPS):
        for dx in range(3):
            for dy in range(3):
                w1r[32 * dy:32 * dy + 32,
                    (3 * g + dx) * 64:(3 * g + dx) * 64 + 64] = \
                    w1[64 * g:64 * g + 64, :, dy, dx].T

    wq_bd = np.zeros((128, 768), np.float32)
    for t in range(2):
        for gg in range(2):
            g = 2 * t + gg
            qlo, qhi = 192 * g, min(640, 192 * g + 192)
            if qlo >= qhi:
                continue
            wq_bd[64 * gg:64 * gg + 64,
                  t * 384 + (qlo - 384 * t):t * 384 + (qhi - 384 * t)] = \
                wqkv2[qlo:qhi, :].T

    def band_weights(base):
        wp_ = np.zeros((B, 128, 320), np.float32)
        for b in range(B):
            yb = int(y[b])
            for h in range(NH):
                c0 = base + 160 * h + 16 * yb
                g = c0 // 192
                t, p0 = g // 2, 64 * (g % 2)
                wp_[b, p0:p0 + 64, 64 * t + 16 * h:64 * t + 16 * h + 16] = \
                    wqkv2[c0:c0 + 16, :].T
        return wp_

    wk = band_weights(640)
    wv = band_weights(1280)
    mask_np = np.zeros((64, 640), np.float32)
    for h in range(NH):
        mask_np[16 * h:16 * h + 16, 160 * h:160 * h + 160] = 1.0
    gma = np.ascontiguousarray(gamma.astype(np.float32).reshape(5, 128).T)
    bta = np.ascontiguousarray(beta.astype(np.float32).reshape(5, 128).T)

    in_maps = []
    for k in range(N_CORES):
        sl = slice(BL * k, BL * k + BL)
        in_maps.append({
            "x_r": np.ascontiguousarray(rep[sl]).astype(np_io),
            "w1r": w1r.astype(np_io),
            "wq": wq_bd.astype(np_io),
            "wk": np.ascontiguousarray(wk[sl]).astype(np_io),
            "wv": np.ascontiguousarray(wv[sl]).astype(np_io),
            "gma": gma,
            "mask": mask_np,
            "bta": bta,
        })
    return in_maps


def _host_assemble(results, y, B):
    o = np.zeros((B, 640, 32, 32), np.float32)
    attn = np.zeros((B, NH, 160, 160), np.float32)
    for k in range(N_CORES):
        r = results[k]
        o[BL * k:BL * k + BL] = r["o_out"].reshape(BL, 640, 32, 32)
        eTf = r["eT_out"]                      # [BL, 64, 640]
        zf = np.transpose(r["z_out"], (0, 2, 1)).reshape(BL, 640)  # [b, ch]
        for lb in range(BL):
            b = BL * k + lb
            yb = int(y[b])
            for h in range(NH):
                zi = zf[lb, 160 * h:160 * h + 160]
                eh = eTf[lb, 16 * h:16 * h + 16, 160 * h:160 * h + 160]
                attn[b, h, :, 16 * yb:16 * yb + 16] = (eh / zi[None, :]).T
    return o, attn


_NC_CACHE = {}


def _get_nc(cfg=DEFAULT_CFG):
    if cfg not in _NC_CACHE:
        _NC_CACHE[cfg] = build_nc(*cfg)
    return _NC_CACHE[cfg]


def run(x, y, w1, b1, gamma, beta, w_qkv, cfg=DEFAULT_CFG, trace=False):
    """Full pipeline; returns ((o, attn), BassKernelResults)."""
    nc, np_io = _get_nc(cfg)
    in_maps = _host_pack(np.asarray(x, np.float32), np.asarray(y),
                         np.asarray(w1, np.float32),
                         np.asarray(gamma, np.float32),
                         np.asarray(beta, np.float32),
                         np.asarray(w_qkv, np.float32), np_io)
    res = run_bass_kernel_spmd(nc, in_maps, list(range(N_CORES)), trace=trace)
    out = _host_assemble(res.results, np.asarray(y), np.asarray(x).shape[0])
    return out, res


def kernel(x, y, w1, b1, gamma, beta, w_qkv):
    out, _ = run(x, y, w1, b1, gamma, beta, w_qkv)
    return out


# revision 10
# speedup vs baseline: 1.5825x; 1.1411x over previous
"""Trainium2 Bass kernel for nn_ClassAttentionBlock (8-core SPMD).

Pipeline per core (8 of 64 batches, data-parallel):
  phase 1: grouped 3x3 conv via K=96 (ci,dy)-packed matmuls over 3 dx taps,
           BN partial stats via bn_stats on PSUM
  sync:    AllReduce of per-channel (sum, sumsq) across the 8 cores
  phase 2: fused BN(scale,bias)+ReLU, grouped 1x1 qkv projections
           (q in transposed [s, ch] layout; k restricted to the class band,
           v band in [ch, s] layout via host-scattered weight tensors)
  phase 3: dotT = k_band^T q over spatial, exp blocks, Z via ones-matmul,
           o = E^T v scaled by 1/Z on the output copy

The class mask never materializes: softmax columns outside the class band
are exactly zero, so only the 16-wide band is computed.  y-dependent channel
selection is folded into host-gathered weight tensors so the NEFF is static.
Host side: shard batch, pack weights, assemble attn from the E/Z outputs.
"""

import numpy as np
import ml_dtypes

import concourse.bass as bass
import concourse.bacc as bacc
import concourse.mybir as mybir
import concourse.tile as tile
from concourse.bass_utils import run_bass_kernel_spmd

N_CORES = 8
BL = 8            # batches per core
GROUPS = 10
S = 1024          # 32*32 spatial
NH = 4
D = 160
SCALE = 640.0 ** (-0.5)
BN_EPS = 1e-5
F32 = mybir.dt.float32
BF16 = mybir.dt.bfloat16

# mm: 'f32' | 'f32r' | 'bf16'   (matmul operand dtype for the big matmuls)
# resident: keep conv output in SBUF (bf16) instead of spilling f32 to DRAM
DEFAULT_CFG = ("f32r", True)


# ----------------------------------------------------------------- builder --
def build_nc(mm: str, resident: bool):
    F32R = mybir.dt.float32r
    # storage dtype for every tensor that feeds a matmul: walrus requires
    # fp32r matmul inputs to be rounded fp32r at the producer, so the whole
    # chain carries the dtype (same 4-byte layout as f32 host-side).
    dt_io = {"f32": F32, "f32r": F32R, "bf16": BF16}[mm]
    np_io = ml_dtypes.bfloat16 if mm == "bf16" else np.float32
    dt_res = BF16 if resident else F32          # conv-out storage

    nc = bacc.Bacc("TRN2", target_bir_lowering=False, debug=False,
                   num_devices=N_CORES)

    xr_in = nc.dram_tensor("x_r", [BL, GROUPS, 96, 1088], dt_io,
                           kind="ExternalInput").ap()
    w1r_in = nc.dram_tensor("w1r", [96, 1920], dt_io, kind="ExternalInput").ap()
    wq_in = nc.dram_tensor("wq", [128, 768], dt_io, kind="ExternalInput").ap()
    wk_in = nc.dram_tensor("wk", [BL, 128, 320], dt_io, kind="ExternalInput").ap()
    wv_in = nc.dram_tensor("wv", [BL, 128, 320], dt_io, kind="ExternalInput").ap()
    gma_in = nc.dram_tensor("gma", [128, 5], F32, kind="ExternalInput").ap()
    mask_in = nc.dram_tensor("mask", [64, 640], F32, kind="ExternalInput").ap()
    bta_in = nc.dram_tensor("bta", [128, 5], F32, kind="ExternalInput").ap()

    o_out = nc.dram_tensor("o_out", [BL, 640, S], F32, kind="ExternalOutput").ap()
    eT_out = nc.dram_tensor("eT_out", [BL, 64, 640], F32, kind="ExternalOutput").ap()
    z_out = nc.dram_tensor("z_out", [BL, 128, 5], F32, kind="ExternalOutput").ap()

    if not resident:
        cv_dram = nc.dram_tensor("cv_spill", [BL, 5, 128, S], F32).ap()

    def mc(ap, n):
        # fp32r runs 1 cyc/row when the moving dim >= 256, else no better
        # than fp32 -- keep plain fp32 numerics for narrow matmuls.
        if mm == "f32r" and n >= 256:
            return ap.bitcast(mybir.dt.float32r)
        return ap

    with tile.TileContext(nc) as tc:
        with (
            tc.tile_pool(name="persist", bufs=1) as pp,
            tc.tile_pool(name="work", bufs=2) as wp,
            tc.tile_pool(name="psA", bufs=2, space="PSUM") as psA,
            tc.tile_pool(name="psB", bufs=4, space="PSUM") as psB,
            tc.tile_pool(name="dram", bufs=1, space="DRAM") as dp,
        ):
            # ---- persistent weights / constants
            w1r_sb = pp.tile([96, 1920], dt_io, tag="w1r")
            nc.sync.dma_start(w1r_sb, w1r_in)
            wq_sb = pp.tile([128, 768], dt_io, tag="wq")
            nc.sync.dma_start(wq_sb, wq_in)
            gma_sb = pp.tile([128, 5], F32, tag="gma")
            nc.sync.dma_start(gma_sb, gma_in)
            bta_sb = pp.tile([128, 5], F32, tag="bta")
            nc.sync.dma_start(bta_sb, bta_in)
            dt_ones = BF16 if mm == "bf16" else F32
            ones_sb = pp.tile([64, 1], dt_ones, tag="ones")
            nc.gpsimd.memset(ones_sb, 1.0)


            # E_T double buffer (fully overwritten per batch) + band mask
            ets = [pp.tile([64, 640], F32, tag=f"et{i}", name=f"et{i}")
                   for i in range(2)]
            if mm != "f32":
                etbs = [pp.tile([64, 640], dt_io, tag=f"etb{i}",
                                name=f"etb{i}") for i in range(2)]
            mask_sb = pp.tile([64, 640], F32, tag="mask")
            nc.sync.dma_start(mask_sb, mask_in)

            # conv-out residency
            if resident:
                convout = [[pp.tile([128, S], dt_res, tag=f"cv{b}_{t}",
                                    name=f"cv{b}_{t}")
                            for t in range(5)] for b in range(BL)]

            stats_sb = pp.tile([128, 480], F32, tag="stats")
            mv_sb = pp.tile([128, 10], F32, tag="mv")
            scale_sb = pp.tile([128, 5], F32, tag="scale")
            bias_sb = pp.tile([128, 5], F32, tag="bias")
            rz_sb = pp.tile([128, 5], F32, tag="rz")

            # per-parity persistent qkv products
            qts = [pp.tile([128, 640], dt_io, tag=f"qt{c}",
                           name=f"qt{c}") for c in range(8)]
            kts = [pp.tile([128, 512], dt_io, tag=f"kt{i}", name=f"kt{i}")
                   for i in range(2)]
            vs = [pp.tile([64, S], dt_io, tag=f"v{i}", name=f"v{i}")
                  for i in range(2)]

            cc_in = dp.tile([128, 10], F32, tag="cc_in")
            cc_out = dp.tile([128, 10], F32, tag="cc_out", addr_space="Shared")

            # ================= phase 1: conv + stats =================
            for b in range(BL):
                for g in range(GROUPS):
                    xr = wp.tile([96, 34 * 32], dt_io, tag="xrep", bufs=3,
                                 name="xr")
                    for dy in range(3):
                        nc.sync.dma_start(xr[32 * dy:32 * dy + 32, :],
                                          xr_in[b, g, 32 * dy:32 * dy + 32, :])

                    ps = psA.tile([64, S], F32, tag="big", name="cps")
                    xv = xr.rearrange("p (h c) -> p h c", c=34)
                    for half in range(2):
                        h0 = 16 * half
                        for dx in (0, 1, 2):
                            lhsT = w1r_sb[:, (3 * g + dx) * 64:(3 * g + dx) * 64 + 64]
                            rhs = xv[:, h0:h0 + 16, dx:dx + 32]
                            nc.tensor.matmul(
                                ps[:, 512 * half:512 * half + 512], lhsT, rhs,
                                start=(dx == 0), stop=(dx == 2))
                    gg, t = g % 2, g // 2
                    for half in range(2):
                        nc.vector.bn_stats(
                            stats_sb[64 * gg:64 * gg + 64,
                                     96 * t + 6 * (2 * b + half):
                                     96 * t + 6 * (2 * b + half) + 6],
                            ps[:, 512 * half:512 * half + 512])
                    if resident:
                        nc.any.tensor_copy(convout[b][t][64 * gg:64 * gg + 64, :], ps)
                    else:
                        cvt = wp.tile([64, S], F32, tag="cvstage", bufs=3,
                                      name="cvstage")
                        nc.scalar.copy(cvt, ps)
                        nc.sync.dma_start(
                            cv_dram[b, t, 64 * gg:64 * gg + 64, :], cvt)

            # ================= stats reduce + BN coefficients ========
            for t in range(5):
                triples = stats_sb[:, 96 * t:96 * t + 96].rearrange(
                    "p (pair eo three) -> p pair eo three", eo=2, three=3)
                nc.vector.bn_aggr(mv_sb[:, 2 * t:2 * t + 2], triples)
            mvv = mv_sb.rearrange("p (t two) -> p two t", two=2)
            means, vars_ = mvv[:, 0, :], mvv[:, 1, :]
            cc_sb = pp.tile([128, 10], F32, tag="cc_sb")
            sq_sb = pp.tile([128, 5], F32, tag="sq")
            nc.vector.tensor_mul(sq_sb, means, means)
            nc.vector.tensor_add(sq_sb, sq_sb, vars_)
            nc.scalar.mul(cc_sb[:, 0:5], means, 8192.0)
            nc.scalar.mul(cc_sb[:, 5:10], sq_sb, 8192.0)
            nc.sync.dma_start(cc_in, cc_sb)
            nc.gpsimd.collective_compute(
                "AllReduce", mybir.AluOpType.add,
                replica_groups=[list(range(N_CORES))],
                ins=[cc_in.opt()], outs=[cc_out.opt()])
            tot_sb = pp.tile([128, 10], F32, tag="tot")
            nc.sync.dma_start(tot_sb, cc_out)
            meanT = pp.tile([128, 5], F32, tag="meanT")
            varT = pp.tile([128, 5], F32, tag="varT")
            nc.scalar.mul(meanT, tot_sb[:, 0:5], 1.0 / 65536.0)
            nc.scalar.mul(varT, tot_sb[:, 5:10], 1.0 / 65536.0)
            m2 = pp.tile([128, 5], F32, tag="m2")
            nc.vector.tensor_mul(m2, meanT, meanT)
            nc.vector.tensor_sub(varT, varT, m2)
            nc.vector.tensor_scalar_add(varT, varT, BN_EPS)
            stdT = pp.tile([128, 5], F32, tag="stdT")
            nc.scalar.sqrt(stdT, varT)
            rstdT = pp.tile([128, 5], F32, tag="rstdT")
            nc.vector.reciprocal(rstdT, stdT)
            nc.vector.tensor_mul(scale_sb, rstdT, gma_sb)
            nc.vector.tensor_mul(m2, meanT, scale_sb)
            nc.vector.tensor_sub(bias_sb, bta_sb, m2)

            # ================= phase 2+3 per batch ===================
            for b in range(BL):
                pb = b % 2
                wk_sb = wp.tile([128, 320], dt_io, tag="wk", name="wk_sb")
                nc.sync.dma_start(wk_sb, wk_in[b])
                wv_sb = wp.tile([128, 320], dt_io, tag="wv", name="wv_sb")
                nc.sync.dma_start(wv_sb, wv_in[b])

                bnt = []
                for t in range(5):
                    bn_ = wp.tile([128, S], dt_io, tag=f"bn{t}", bufs=2,
                                  name=f"bn_{t}")
                    if resident:
                        src = convout[b][t]
                    else:
                        src = wp.tile([128, S], F32, tag="rl", bufs=3, name="rl")
                        nc.sync.dma_start(src, cv_dram[b, t])
                    nc.scalar.activation(
                        bn_, src, mybir.ActivationFunctionType.Relu,
                        bias=bias_sb[:, t:t + 1], scale=scale_sb[:, t:t + 1])
                    bnt.append(bn_)

                # q (transposed layout)  psum [s_chunk=128, 640]
                for sc in range(8):
                    qp = psA.tile([128, 640], F32, tag="big", name="qps")
                    sl = slice(128 * sc, 128 * sc + 128)
                    nc.tensor.matmul(qp[:, 0:384], mc(bnt[0][:, sl], 384),
                                     mc(wq_sb[:, 0:384], 384),
                                     start=True, stop=True)
                    nc.tensor.matmul(qp[:, 384:512], bnt[1][:, sl],
                                     wq_sb[:, 384:512], start=True, stop=True)
                    nc.tensor.matmul(qp[:, 512:640], bnt[1][:, sl],
                                     wq_sb[:, 512:640], start=True, stop=True)
                    nc.any.tensor_copy(qts[sc], qp)
                # k band (transposed layout)
                for sc in range(8):
                    kp = psB.tile([128, 64], F32, tag="small", name="kps")
                    sl = slice(128 * sc, 128 * sc + 128)
                    for t in (1, 2, 3):
                        nc.tensor.matmul(kp, bnt[t][:, sl],
                                         wk_sb[:, 64 * t:64 * t + 64],
                                         start=(t == 1), stop=(t == 3))
                    nc.any.tensor_copy(kts[pb][:, 64 * sc:64 * sc + 64], kp)
                # v band (channel layout)
                vp = psA.tile([64, S], F32, tag="big", name="vps")
                for t in range(5):
                    for n_ in range(2):
                        nc.tensor.matmul(
                            vp[:, 512 * n_:512 * n_ + 512],
                            mc(wv_sb[:, 64 * t:64 * t + 64], 512),
                            mc(bnt[t][:, 512 * n_:512 * n_ + 512], 512),
                            start=(t == 0), stop=(t == 4))
                nc.any.tensor_copy(vs[pb], vp)

                # dotT [64 j, 640 i]
                dpm = psA.tile([64, 640], F32, tag="big", name="dps")
                for sc in range(8):
                    ksl = kts[pb][:, 64 * sc:64 * sc + 64]
                    nc.tensor.matmul(dpm[:, 0:512], ksl,
                                     qts[sc][:, 0:512],
                                     start=(sc == 0), stop=(sc == 7))
                    nc.tensor.matmul(dpm[:, 512:640], ksl,
                                     qts[sc][:, 512:640],
                                     start=(sc == 0), stop=(sc == 7))
                # exp on the full tile (engines need 32-aligned partition
                # bases, so per-band slices at partition 16h are illegal);
                # off-band junk is zeroed by the precomputed band mask.
                efull = wp.tile([64, 640], F32, tag="efull", name="efull")
                nc.scalar.activation(efull, dpm,
                                     mybir.ActivationFunctionType.Exp,
                                     scale=SCALE)
                et = ets[pb]
                nc.vector.tensor_mul(et, efull, mask_sb)
                nc.sync.dma_start(eT_out[b], et)
                if mm != "f32":
                    emm = etbs[pb]
                    nc.vector.tensor_copy(emm, et)
                else:
                    emm = et
                # Z via ones-matmul -> [128, 5] per chunk
                zp = psB.tile([128, 8], F32, tag="small", name="zps")
                ez = (et if mm == "f32r" else emm)
                for c in range(5):
                    nc.tensor.matmul(zp[:, c:c + 1],
                                     ez[:, 128 * c:128 * c + 128], ones_sb,
                                     start=True, stop=True)
                nc.vector.reciprocal(rz_sb, zp[:, 0:5])
                zst = wp.tile([128, 5], F32, tag="zst", name="zst")
                nc.vector.tensor_copy(zst, zp[:, 0:5])
                nc.sync.dma_start(z_out[b], zst)
                # o = E^T v * (1/Z)
                for c in range(5):
                    for n_ in range(2):
                        op_ = psB.tile([128, 512], F32, tag="small", name="ops")
                        nc.tensor.matmul(op_,
                                         mc(emm[:, 128 * c:128 * c + 128], 512),
                                         mc(vs[pb][:, 512 * n_:512 * n_ + 512], 512),
                                         start=True, stop=True)
                        ost = wp.tile([128, 512], F32, tag="ost", bufs=3,
                                      name="ost")
                        nc.scalar.mul(ost, op_, rz_sb[:, c:c + 1])
                        nc.sync.dma_start(
                            o_out[b, 128 * c:128 * c + 128,
                                  512 * n_:512 * n_ + 512], ost)

    nc.compile()
    return nc, np_io


# ------------------------------------------------------------- host packing --
def _host_pack(x, y, w1, gamma, beta, w_qkv, np_io):
    B = x.shape[0]
    # padded dy-replicated conv input: rep[b, g, 32*dy+ci, r, 1+w]
    #   = x[b, 32g+ci, r+dy-1, w], zero padding elsewhere
    x4 = x.reshape(B, GROUPS, 32, 32, 32)
    rep = np.zeros((B, GROUPS, 3, 32, 32, 34), np.float32)
    rep[:, :, 0, :, 1:32, 1:33] = x4[:, :, :, 0:31, :]
    rep[:, :, 1, :, :, 1:33] = x4
    rep[:, :, 2, :, 0:31, 1:33] = x4[:, :, :, 1:32, :]
    rep = rep.reshape(B, GROUPS, 96, 1088)
    wqkv2 = w_qkv[:, :, 0, 0].astype(np.float32)

    w1r = np.zeros((96, 1920), np.float32)
    for g in range(GROUPS):
        for dx in range(3):
            for dy in range(3):
                w1r[32 * dy:32 * dy + 32,
                    (3 * g + dx) * 64:(3 * g + dx) * 64 + 64] = \
                    w1[64 * g:64 * g + 64, :, dy, dx].T

    wq_bd = np.zeros((128, 768), np.float32)
    for t in range(2):
        for gg in range(2):
            g = 2 * t + gg
            qlo, qhi = 192 * g, min(640, 192 * g + 192)
            if qlo >= qhi:
                continue
            wq_bd[64 * gg:64 * gg + 64,
                  t * 384 + (qlo - 384 * t):t * 384 + (qhi - 384 * t)] = \
                wqkv2[qlo:qhi, :].T

    def band_weights(base):
        wp_ = np.zeros((B, 128, 320), np.float32)
        for b in range(B):
            yb = int(y[b])
            for h in range(NH):
                c0 = base + 160 * h + 16 * yb
                g = c0 // 192
                t, p0 = g // 2, 64 * (g % 2)
                wp_[b, p0:p0 + 64, 64 * t + 16 * h:64 * t + 16 * h + 16] = \
                    wqkv2[c0:c0 + 16, :].T
        return wp_

    wk = band_weights(640)
    wv = band_weights(1280)
    mask_np = np.zeros((64, 640), np.float32)
    for h in range(NH):
        mask_np[16 * h:16 * h + 16, 160 * h:160 * h + 160] = 1.0
    gma = np.ascontiguousarray(gamma.astype(np.float32).reshape(5, 128).T)
    bta = np.ascontiguousarray(beta.astype(np.float32).reshape(5, 128).T)

    in_maps = []
    for k in range(N_CORES):
        sl = slice(BL * k, BL * k + BL)
        in_maps.append({
            "x_r": np.ascontiguousarray(rep[sl]).astype(np_io),
            "w1r": w1r.astype(np_io),
            "wq": wq_bd.astype(np_io),
            "wk": np.ascontiguousarray(wk[sl]).astype(np_io),
            "wv": np.ascontiguousarray(wv[sl]).astype(np_io),
            "gma": gma,
            "mask": mask_np,
            "bta": bta,
        })
    return in_maps


def _host_assemble(results, y, B):
    o = np.zeros((B, 640, 32, 32), np.float32)
    attn = np.zeros((B, NH, 160, 160), np.float32)
    for k in range(N_CORES):
        r = results[k]
        o[BL * k:BL * k + BL] = r["o_out"].reshape(BL, 640, 32, 32)
        eTf = r["eT_out"]                      # [BL, 64, 640]
        zf = np.transpose(r["z_out"], (0, 2, 1)).reshape(BL, 640)  # [b, ch]
        for lb in range(BL):
            b = BL * k + lb
            yb = int(y[b])
            for h in range(NH):
                zi = zf[lb, 160 * h:160 * h + 160]
                eh = eTf[lb, 16 * h:16 * h + 16, 160 * h:160 * h + 160]
                attn[b, h, :, 16 * yb:16 * yb + 16] = (eh / zi[None, :]).T
    return o, attn


_NC_CACHE = {}


def _get_nc(cfg=DEFAULT_CFG):
    if cfg not in _NC_CACHE:
        _NC_CACHE[cfg] = build_nc(*cfg)
    return _NC_CACHE[cfg]


def run(x, y, w1, b1, gamma, beta, w_qkv, cfg=DEFAULT_CFG, trace=False):
    """Full pipeline; returns ((o, attn), BassKernelResults)."""
    nc, np_io = _get_nc(cfg)
    in_maps = _host_pack(np.asarray(x, np.float32), np.asarray(y),
                         np.asarray(w1, np.float32),
                         np.asarray(gamma, np.float32),
                         np.asarray(beta, np.float32),
                         np.asarray(w_qkv, np.float32), np_io)
    res = run_bass_kernel_spmd(nc, in_maps, list(range(N_CORES)), trace=trace)
    out = _host_assemble(res.results, np.asarray(y), np.asarray(x).shape[0])
    return out, res


def kernel(x, y, w1, b1, gamma, beta, w_qkv):
    out, _ = run(x, y, w1, b1, gamma, beta, w_qkv)
    return out
